# revision 23
# baseline (speedup 1.0000x reference)
"""ABC attention (Attention with Bounded-memory Control) on 8 TRN2 NeuronCores.

Sharding: one (batch, head) pair per core (B=2 x H=4 = 8 shards). Each core
computes its head's q/k/v/g projections, the chunked linear-attention ABC
recurrence (chunk C=128, [M,D] cumulative slot states instead of the O(T^2)
attention matrices), the gated RMSNorm epilogue, and its partial o_proj
contribution [T, D]. The host sums the 4 per-head partials of each batch.

Self-contained: hardcodes all shapes; inputs arrive as the full unsharded
tensors keyed as in setup_inputs().
"""
import contextlib
import math

import numpy as np
import ml_dtypes

# ---------------------------------------------------------------- constants
B, T, D, H = 2, 2048, 1024, 4
DK = D // H            # 256 head dim
HALF = DK // 2         # 128 rope half
M = 64                 # slots
LOW = 16               # gate low rank
C = 128                # time chunk
NCH = T // C           # 16 chunks
CLAMP = 32.0
EPS = 1e-5
SCALE = DK ** -0.5
N_CORES = 8

BF16 = ml_dtypes.bfloat16

_CACHE = {}


# ---------------------------------------------------------------- tile patch
def _install_tile_patch():
    """This walrus build rejects >1 sync wait per instruction; the stock Tile
    tail drain carries one wait per live processor. Split the extra waits onto
    single-wait SP nops before the barrier + semaphore clear."""
    import concourse.tile as tile
    from concourse import mybir
    from concourse.vector_clock import ScopedClock

    if getattr(tile.TileContext, "_abc_patch", False):
        return
    tile.TileContext._abc_patch = True

    def _drain_and_barrier_split(self, tick_clock, wait_clock):
        drain_inst = self.nc.sync.drain()
        wait_clock.add_sem_waits(
            drain_inst.ins, ScopedClock({None: tick_clock.global_clock})
        )
        si = drain_inst.ins.sync_info
        waits = list(si.on_wait) if si is not None else []
        if len(waits) > 1:
            drain_inst.ins.sync_info = mybir.SyncInfo(on_wait=[waits[0]], on_update=[])
            for w in waits[1:]:
                nop = self.nc.sync.nop()
                nop.ins.sync_info = mybir.SyncInfo(on_wait=[w], on_update=[])
        self.nc.all_engine_barrier()
        assert self.sems is not None
        popped = self.nc._tile_sem_poison_stack.pop()
        assert popped is self._sem_poison
        self.nc.clear_and_free_semaphores(list(self.sems.allocated().values()))
        self.nc.all_engine_barrier()

    tile.TileContext._drain_and_barrier = _drain_and_barrier_split


def _legalize_waits(nc):
    """This walrus build accepts at most one sync wait per regular instruction
    (two for EventSemaphore). Hoist overflow waits onto fresh NoOps inserted
    just before the offending instruction in the same engine stream."""
    from concourse import mybir

    n = 0
    for blk in nc.m.functions[0].blocks:
        rebuilt = []
        changed = False
        for inst in blk.instructions:
            si = inst.sync_info
            waits = list(si.on_wait) if si is not None else []
            cap = 2 if isinstance(inst, mybir.InstEventSemaphore) else 1
            if len(waits) > cap:
                changed = True
                for w in waits[cap:]:
                    nop = mybir.InstNoOp(name=f"WSPLIT-{n}", ins=[], outs=[])
                    n += 1
                    nop.engine = inst.engine
                    nop.sync_info = mybir.SyncInfo(on_wait=[w], on_update=[])
                    rebuilt.append(nop)
                inst.sync_info = mybir.SyncInfo(on_wait=waits[:cap],
                                                on_update=list(si.on_update))
            rebuilt.append(inst)
        if changed:
            blk.instructions = rebuilt


# ---------------------------------------------------------------- host tables
def _rope_tables():
    inv_freq = 1.0 / (10000.0 ** (np.arange(HALF, dtype=np.float64) / HALF))
    ang = np.arange(T, dtype=np.float64)[:, None] * inv_freq[None, :]  # [T, HALF]
    cos = np.cos(ang).astype(np.float32)
    sin = np.sin(ang).astype(np.float32)
    # D-layout [HALF, T]: row i, col t
    cosT = np.ascontiguousarray(cos.T)
    sinT = np.ascontiguousarray(sin.T)
    # T-layout packed per chunk: [128, NCH*128]; (p, c*128+j) = table[c*128+p, j]
    cosn = np.ascontiguousarray(
        cos.reshape(NCH, C, HALF).transpose(1, 0, 2).reshape(C, NCH * HALF))
    sinn = np.ascontiguousarray(
        sin.reshape(NCH, C, HALF).transpose(1, 0, 2).reshape(C, NCH * HALF))
    return cosT, sinT, cosn, sinn


# ---------------------------------------------------------------- builder
def _build_nc():
    _install_tile_patch()
    import concourse.bass as bass
    import concourse.tile as tile
    from concourse import mybir

    f32 = mybir.dt.float32
    bf = mybir.dt.bfloat16
    AX = mybir.AxisListType
    AF = mybir.ActivationFunctionType
    ALU = mybir.AluOpType

    nc = bass.Bass("TRN2", target_bir_lowering=False, debug=False,
                   num_devices=N_CORES)

    # ---- dram parameters (per-core shards supplied via in_maps)
    xT_d = nc.dram_tensor("xT", [D, T], bf, kind="ExternalInput")
    wq_d = nc.dram_tensor("wq", [D, DK], bf, kind="ExternalInput")
    wk_d = nc.dram_tensor("wk", [D, DK], bf, kind="ExternalInput")
    wv_d = nc.dram_tensor("wv", [D, DK], bf, kind="ExternalInput")
    wg_d = nc.dram_tensor("wg", [D, DK], bf, kind="ExternalInput")
    wo_d = nc.dram_tensor("wo", [DK, D], bf, kind="ExternalInput")
    w1_d = nc.dram_tensor("w1", [D, 2 * LOW], bf, kind="ExternalInput")
    w2k_d = nc.dram_tensor("w2k", [LOW, M], bf, kind="ExternalInput")
    w2v_d = nc.dram_tensor("w2v", [LOW, M], bf, kind="ExternalInput")
    bkr_d = nc.dram_tensor("bkr", [1, M], bf, kind="ExternalInput")
    bvr_d = nc.dram_tensor("bvr", [1, M], bf, kind="ExternalInput")
    bvc_d = nc.dram_tensor("bvc", [M, 1], f32, kind="ExternalInput")
    gnw_d = nc.dram_tensor("gnw", [C, 2], f32, kind="ExternalInput")
    out_d = nc.dram_tensor("out", [T, D], f32, kind="ExternalOutput")

    # ---- inline constants (same on every core)
    cosT_h, sinT_h, cosn_h, sinn_h = _rope_tables()
    mask_h = np.triu(np.ones((C, C), dtype=np.float32))   # [s,t] = 1 if t >= s
    ident_h = np.eye(C, dtype=np.float32)
    cosT_c = nc.inline_tensor(cosT_h.astype(BF16), name="cosT")
    sinT_c = nc.inline_tensor(sinT_h.astype(BF16), name="sinT")
    cosn_c = nc.inline_tensor(cosn_h.astype(BF16), name="cosn")
    sinn_c = nc.inline_tensor(sinn_h.astype(BF16), name="sinn")
    mask_f_c = nc.inline_tensor(mask_h, name="mask_f")
    mask_b_c = nc.inline_tensor(mask_h.astype(BF16), name="mask_b")
    ident_c = nc.inline_tensor(ident_h.astype(BF16), name="ident")
    ones_col_c = nc.inline_tensor(np.ones((C, 1), dtype=BF16), name="ones_col")
    ones_row_f_c = nc.inline_tensor(np.ones((1, C), dtype=np.float32), name="ones_row_f")
    ones_row_b_c = nc.inline_tensor(np.ones((1, C), dtype=BF16), name="ones_row_b")

    with tile.TileContext(nc) as tc, contextlib.ExitStack() as es:
        cp = es.enter_context(tc.tile_pool(name="const", bufs=1))
        wp = es.enter_context(tc.tile_pool(name="work", bufs=3))
        op_ = es.enter_context(tc.tile_pool(name="outstage", bufs=3))
        ps_big = es.enter_context(tc.tile_pool(name="psbig", bufs=3, space="PSUM"))
        ps_st = es.enter_context(tc.tile_pool(name="psst", bufs=1, space="PSUM"))
        ps_lz = es.enter_context(tc.tile_pool(name="pslz", bufs=2, space="PSUM"))
        ps_pt = es.enter_context(tc.tile_pool(name="pspt", bufs=1, space="PSUM"))
        ps_ot = es.enter_context(tc.tile_pool(name="psot", bufs=1, space="PSUM"))

        dma = nc.sync.dma_start

        # ================= constant / persistent SBUF =================
        xT = cp.tile([C, 8 * T], bf)            # x^T: ktile kt at cols [kt*T, (kt+1)*T)
        dma(xT[:].rearrange("p (n t) -> p n t", n=8),
            xT_d.ap().rearrange("(n p) t -> p n t", p=C))
        w_sb = {}
        for name, d_t in (("wq", wq_d), ("wk", wk_d), ("wv", wv_d), ("wg", wg_d)):
            t_ = cp.tile([C, 8 * DK], bf, tag=f"w_{name}")  # ktile kt at [kt*DK,)
            dma(t_[:].rearrange("p (n c) -> p n c", n=8),
                d_t.ap().rearrange("(n p) c -> p n c", p=C))
            w_sb[name] = t_
        wo = cp.tile([C, 2 * D], bf)            # half hh at cols [hh*D, ...)
        dma(wo[:].rearrange("p (n c) -> p n c", n=2),
            wo_d.ap().rearrange("(n p) c -> p n c", p=C))
        w1 = cp.tile([C, 8 * 2 * LOW], bf)
        dma(w1[:].rearrange("p (n c) -> p n c", n=8),
            w1_d.ap().rearrange("(n p) c -> p n c", p=C))
        w2k = cp.tile([LOW, M], bf)
        dma(w2k[:], w2k_d.ap())
        w2v = cp.tile([LOW, M], bf)
        dma(w2v[:], w2v_d.ap())
        bkr = cp.tile([1, M], bf); dma(bkr[:], bkr_d.ap())
        bvr = cp.tile([1, M], bf); dma(bvr[:], bvr_d.ap())
        bvc = cp.tile([M, 1], f32); dma(bvc[:], bvc_d.ap())
        gnw = cp.tile([C, 2], f32)
        dma(gnw[:], gnw_d.ap())

        cosT = cp.tile([C, T], bf); dma(cosT[:], cosT_c.ap())
        sinT = cp.tile([C, T], bf); dma(sinT[:], sinT_c.ap())
        cosn = cp.tile([C, T], bf); dma(cosn[:], cosn_c.ap())
        sinn = cp.tile([C, T], bf); dma(sinn[:], sinn_c.ap())
        mask_f = cp.tile([C, C], f32); dma(mask_f[:], mask_f_c.ap())
        mask_b = cp.tile([C, C], bf); dma(mask_b[:], mask_b_c.ap())
        ident = cp.tile([C, C], bf); dma(ident[:], ident_c.ap())
        ones_col = cp.tile([C, 1], bf); dma(ones_col[:], ones_col_c.ap())
        ones_row_b = cp.tile([1, C], bf); dma(ones_row_b[:], ones_row_b_c.ap())

        qT0 = cp.tile([C, T], bf); qT1 = cp.tile([C, T], bf)     # rope(q)^T halves
        kT0 = cp.tile([C, T], bf); kT1 = cp.tile([C, T], bf)     # rope(k)^T halves
        k_nat = cp.tile([C, NCH * DK], bf)     # rope(k) chunk tch at cols [tch*DK,)
        v_nat = cp.tile([C, NCH * DK], bf)
        gT0 = cp.tile([C, T], bf); gT1 = cp.tile([C, T], bf)
        yk = cp.tile([LOW, T], bf)
        yv = cp.tile([LOW, T], bf)
        sk_all = cp.tile([C, NCH * M], bf)     # clamped slot logits -> exp in place
        sv_all = cp.tile([C, NCH * M], bf)
        svT_all = cp.tile([M, T], bf)
        wk_e, wv_e, wvT_e = sk_all, sv_all, svT_all
        kst_pfx = cp.tile([C, NCH * C], bf)    # KstateT prefix: chunk c, half hh at
        #                                        cols [c*128 + hh*64)
        vst_pfx = cp.tile([M, NCH * DK], bf)   # Vstate prefix
        zk_pfx = cp.tile([1, NCH * M], bf)
        zv_pfx = cp.tile([1, NCH * M], bf)
        kst_acc = cp.tile([C, C], f32)
        vst_acc = cp.tile([M, DK], f32)
        z_acc = cp.tile([1, 2 * M], f32)
        oT0 = cp.tile([C, T], bf); oT1 = cp.tile([C, T], bf)
        ss_all = cp.tile([1, T], bf)
        rnorm = cp.tile([1, T], bf)

        MMf = dict(start=True, stop=True)

        def rope_pair(dst0, dst1, ps0, ps1, cos_t, sin_t, col, n):
            """dst[0/1][:, col:col+n] = rot(ps0, ps1) with [128,n] tables."""
            t1 = wp.tile([C, n], f32, tag="ropeA")
            t2 = wp.tile([C, n], f32, tag="ropeB")
            nc.vector.tensor_mul(t1[:], ps0, cos_t)
            nc.vector.tensor_mul(t2[:], ps1, sin_t)
            nc.vector.tensor_sub(dst0[:, col:col + n], t1[:], t2[:])
            nc.vector.tensor_mul(t1[:], ps1, cos_t)
            nc.vector.tensor_mul(t2[:], ps0, sin_t)
            nc.vector.tensor_add(dst1[:, col:col + n], t1[:], t2[:])

        # ================= phase A: projections =================
        # q^T, k^T, g^T (D-layout) + ykv, per 512-wide time slab
        for j in range(T // 512):
            c0 = j * 512
            # ykv: [32, 512]
            ps_y = ps_big.tile([2 * LOW, 512], f32, tag="big")
            for kt in range(8):
                nc.tensor.matmul(ps_y[:], w1[:, kt * 2 * LOW:(kt + 1) * 2 * LOW],
                                 xT[:, kt * T + c0:kt * T + c0 + 512],
                                 start=(kt == 0), stop=(kt == 7))
            # partition-16 source reads are illegal on DVE; stage to SBUF then
            # shift partitions via SBUF->SBUF DMA
            ystg = wp.tile([2 * LOW, 512], bf, tag="ystg")
            nc.vector.tensor_copy(ystg[:], ps_y[:])
            dma(yk[0:LOW, c0:c0 + 512], ystg[0:LOW, :])
            dma(yv[0:LOW, c0:c0 + 512], ystg[LOW:2 * LOW, :])

            for wname, d0, d1 in (("wq", qT0, qT1), ("wk", kT0, kT1)):
                w_ = w_sb[wname]
                psA = ps_big.tile([C, 512], f32, tag="big")
                psB = ps_big.tile([C, 512], f32, tag="big")
                for kt in range(8):
                    nc.tensor.matmul(psA[:], w_[:, kt * DK:kt * DK + HALF],
                                     xT[:, kt * T + c0:kt * T + c0 + 512],
                                     start=(kt == 0), stop=(kt == 7))
                for kt in range(8):
                    nc.tensor.matmul(psB[:], w_[:, kt * DK + HALF:(kt + 1) * DK],
                                     xT[:, kt * T + c0:kt * T + c0 + 512],
                                     start=(kt == 0), stop=(kt == 7))
                rope_pair(d0, d1, psA[:], psB[:],
                          cosT[:, c0:c0 + 512], sinT[:, c0:c0 + 512], c0, 512)
            # g^T: no rope
            for hh, dst in ((0, gT0), (1, gT1)):
                psG = ps_big.tile([C, 512], f32, tag="big")
                for kt in range(8):
                    nc.tensor.matmul(psG[:], w_sb["wg"][:, kt * DK + hh * HALF:
                                                        kt * DK + (hh + 1) * HALF],
                                     xT[:, kt * T + c0:kt * T + c0 + 512],
                                     start=(kt == 0), stop=(kt == 7))
                nc.vector.tensor_copy(dst[:, c0:c0 + 512], psG[:])

        # k, v in natural T-layout, per 128 chunk
        for tch in range(NCH):
            c0 = tch * C
            psK = ps_big.tile([C, DK], f32, tag="big")
            for kt in range(8):
                nc.tensor.matmul(psK[:], xT[:, kt * T + c0:kt * T + c0 + C],
                                 w_sb["wk"][:, kt * DK:(kt + 1) * DK],
                                 start=(kt == 0), stop=(kt == 7))
            # T-layout rope into k_nat cols [tch*DK, tch*DK+DK)
            tc_ = cosn[:, tch * HALF:(tch + 1) * HALF]
            ts_ = sinn[:, tch * HALF:(tch + 1) * HALF]
            t1 = wp.tile([C, HALF], f32, tag="ropeC")
            t2 = wp.tile([C, HALF], f32, tag="ropeD")
            nc.vector.tensor_mul(t1[:], psK[:, 0:HALF], tc_)
            nc.vector.tensor_mul(t2[:], psK[:, HALF:DK], ts_)
            nc.vector.tensor_sub(k_nat[:, tch * DK:tch * DK + HALF], t1[:], t2[:])
            nc.vector.tensor_mul(t1[:], psK[:, HALF:DK], tc_)
            nc.vector.tensor_mul(t2[:], psK[:, 0:HALF], ts_)
            nc.vector.tensor_add(k_nat[:, tch * DK + HALF:(tch + 1) * DK], t1[:], t2[:])

            psV = ps_big.tile([C, DK], f32, tag="big")
            for kt in range(8):
                nc.tensor.matmul(psV[:], xT[:, kt * T + c0:kt * T + c0 + C],
                                 w_sb["wv"][:, kt * DK:(kt + 1) * DK],
                                 start=(kt == 0), stop=(kt == 7))
            nc.vector.tensor_copy(v_nat[:, tch * DK:(tch + 1) * DK], psV[:])

        # ---- slot gate logits (clamped), then batched exp
        for tch in range(NCH):
            c0 = tch * C
            ps_s = ps_lz.tile([C, 2 * M], f32, tag="lz")
            nc.tensor.matmul(ps_s[:, 0:M], yk[:, c0:c0 + C], w2k[:],
                             start=True, stop=False)
            nc.tensor.matmul(ps_s[:, 0:M], ones_row_b[:], bkr[:],
                             start=False, stop=True)
            nc.tensor.matmul(ps_s[:, M:2 * M], yv[:, c0:c0 + C], w2v[:],
                             start=True, stop=False)
            nc.tensor.matmul(ps_s[:, M:2 * M], ones_row_b[:], bvr[:],
                             start=False, stop=True)
            nc.vector.tensor_scalar(sk_all[:, tch * M:(tch + 1) * M], ps_s[:, 0:M],
                                    -CLAMP, CLAMP, ALU.max, ALU.min)
            nc.vector.tensor_scalar(sv_all[:, tch * M:(tch + 1) * M], ps_s[:, M:2 * M],
                                    -CLAMP, CLAMP, ALU.max, ALU.min)
            ps_t = ps_pt.tile([M, C], f32, tag="pt")
            nc.tensor.matmul(ps_t[:], w2v[:], yv[:, c0:c0 + C], **MMf)
            svt_tmp = wp.tile([M, C], f32, tag="svtmp")
            nc.vector.tensor_scalar(svt_tmp[:], ps_t[:], bvc[:, 0:1], -CLAMP,
                                    ALU.add, ALU.max)
            nc.vector.tensor_scalar_min(svT_all[:, c0:c0 + C], svt_tmp[:], CLAMP)
        nc.scalar.activation(wk_e[:], sk_all[:], AF.Exp)
        nc.scalar.activation(wv_e[:], sv_all[:], AF.Exp)
        nc.scalar.activation(wvT_e[:], svT_all[:], AF.Exp)

        # ---- state prefixes (sequential over chunks)
        for tch in range(NCH):
            wk_c = wk_e[:, tch * M:(tch + 1) * M]
            wv_c = wv_e[:, tch * M:(tch + 1) * M]
            kd = ps_st.tile([C, C], f32, tag="st")
            for hh in range(2):
                nc.tensor.matmul(kd[:, hh * M:(hh + 1) * M],
                                 k_nat[:, tch * DK + hh * C:tch * DK + (hh + 1) * C],
                                 wk_c, **MMf)
            vd = ps_ot.tile([M, DK], f32, tag="ot")
            nc.tensor.matmul(vd[:], wv_c, v_nat[:, tch * DK:(tch + 1) * DK], **MMf)
            zd = ps_lz.tile([1, 2 * M], f32, tag="lz")
            nc.tensor.matmul(zd[:, 0:M], ones_col[:], wk_c, **MMf)
            nc.tensor.matmul(zd[:, M:2 * M], ones_col[:], wv_c, **MMf)
            if tch == 0:
                nc.vector.tensor_copy(kst_acc[:], kd[:])
                nc.vector.tensor_copy(vst_acc[:], vd[:])
                nc.vector.tensor_copy(z_acc[:], zd[:])
            else:
                nc.vector.tensor_add(kst_acc[:], kst_acc[:], kd[:])
                nc.vector.tensor_add(vst_acc[:], vst_acc[:], vd[:])
                nc.vector.tensor_add(z_acc[:], z_acc[:], zd[:])
            nc.vector.tensor_copy(kst_pfx[:, tch * C:(tch + 1) * C], kst_acc[:])
            nc.vector.tensor_copy(vst_pfx[:, tch * DK:(tch + 1) * DK], vst_acc[:])
            nc.vector.tensor_copy(zk_pfx[:, tch * M:(tch + 1) * M], z_acc[:, 0:M])
            nc.vector.tensor_copy(zv_pfx[:, tch * M:(tch + 1) * M], z_acc[:, M:2 * M])

        # ================= phase B: per-chunk attention =================
        for ch in range(NCH):
            c0 = ch * C
            wk_c = wk_e[:, ch * M:(ch + 1) * M]
            wv_c = wv_e[:, ch * M:(ch + 1) * M]
            # ST[s,t] = sum_d k^T[d,s] q^T[d,t]
            st_ps = ps_st.tile([C, C], f32, tag="st")
            nc.tensor.matmul(st_ps[:], kT0[:, c0:c0 + C], qT0[:, c0:c0 + C],
                             start=True, stop=False)
            nc.tensor.matmul(st_ps[:], kT1[:, c0:c0 + C], qT1[:, c0:c0 + C],
                             start=False, stop=True)
            st_m = wp.tile([C, C], bf, tag="stm")
            nc.vector.tensor_mul(st_m[:], st_ps[:], mask_f[:])

            lz = ps_lz.tile([C, 3 * M], f32, tag="lz")
            # logits[t,m] = ST_m^T wk + q_t . Kstate
            nc.tensor.matmul(lz[:, 0:M], st_m[:], wk_c, start=True, stop=(ch == 0))
            if ch > 0:
                pcol = (ch - 1) * C
                nc.tensor.matmul(lz[:, 0:M], qT0[:, c0:c0 + C],
                                 kst_pfx[:, pcol:pcol + M], start=False, stop=False)
                nc.tensor.matmul(lz[:, 0:M], qT1[:, c0:c0 + C],
                                 kst_pfx[:, pcol + M:pcol + 2 * M],
                                 start=False, stop=True)
            # zk, zv: within-chunk cumulative sums + carried state
            pm = (ch - 1) * M
            nc.tensor.matmul(lz[:, M:2 * M], mask_b[:], wk_c,
                             start=True, stop=(ch == 0))
            if ch > 0:
                nc.tensor.matmul(lz[:, M:2 * M], ones_row_b[:],
                                 zk_pfx[:, pm:pm + M], start=False, stop=True)
            nc.tensor.matmul(lz[:, 2 * M:3 * M], mask_b[:], wv_c,
                             start=True, stop=(ch == 0))
            if ch > 0:
                nc.tensor.matmul(lz[:, 2 * M:3 * M], ones_row_b[:],
                                 zv_pfx[:, pm:pm + M], start=False, stop=True)

            rzk = wp.tile([C, M], f32, tag="rzk")
            nc.vector.reciprocal(rzk[:], lz[:, M:2 * M])
            rzv = wp.tile([C, M], f32, tag="rzv")
            nc.vector.reciprocal(rzv[:], lz[:, 2 * M:3 * M])
            l1 = wp.tile([C, M], f32, tag="l1")
            nc.vector.tensor_mul(l1[:], lz[:, 0:M], rzk[:])
            mx = wp.tile([C, 1], f32, tag="mx")
            nc.vector.tensor_reduce(mx[:], l1[:], axis=AX.X, op=ALU.max)
            nmx = wp.tile([C, 1], f32, tag="nmx")
            nc.vector.tensor_scalar_mul(nmx[:], mx[:], -SCALE)
            e = wp.tile([C, M], f32, tag="e")
            rs = wp.tile([C, 1], f32, tag="rs")
            nc.scalar.activation(e[:], l1[:], AF.Exp, bias=nmx[:, 0:1], scale=SCALE,
                                 accum_out=rs[:, 0:1])
            rden = wp.tile([C, 1], f32, tag="rden")
            nc.vector.reciprocal(rden[:], rs[:])
            ptmp = wp.tile([C, M], f32, tag="ptmp")
            nc.vector.tensor_scalar_mul(ptmp[:], e[:], rden[:, 0:1])
            pt_bf = wp.tile([C, M], bf, tag="ptbf")
            nc.vector.tensor_mul(pt_bf[:], ptmp[:], rzv[:])

            ptT_ps = ps_pt.tile([M, C], bf, tag="pt")
            nc.tensor.transpose(ptT_ps[:], pt_bf[:], ident[:])
            ptT = wp.tile([M, C], bf, tag="ptT")
            nc.vector.tensor_copy(ptT[:], ptT_ps[:])

            at_ps = ps_st.tile([C, C], f32, tag="st")
            nc.tensor.matmul(at_ps[:], wvT_e[:, c0:c0 + C], ptT[:], **MMf)
            at_m = wp.tile([C, C], bf, tag="atm")
            nc.vector.tensor_mul(at_m[:], at_ps[:], mask_f[:])

            ot = ps_ot.tile([C, 2 * C], f32, tag="ot")
            for hh in range(2):
                nc.tensor.matmul(ot[:, hh * C:(hh + 1) * C],
                                 v_nat[:, ch * DK + hh * C:ch * DK + (hh + 1) * C],
                                 at_m[:], start=True, stop=(ch == 0))
                if ch > 0:
                    nc.tensor.matmul(ot[:, hh * C:(hh + 1) * C],
                                     vst_pfx[:, (ch - 1) * DK + hh * C:
                                             (ch - 1) * DK + (hh + 1) * C],
                                     ptT[:], start=False, stop=True)
            nc.vector.tensor_copy(oT0[:, c0:c0 + C], ot[:, 0:C])
            nc.vector.tensor_copy(oT1[:, c0:c0 + C], ot[:, C:2 * C])
            sq = wp.tile([C, 2 * C], bf, tag="sq")
            nc.vector.tensor_mul(sq[:, 0:C], oT0[:, c0:c0 + C], oT0[:, c0:c0 + C])
            nc.vector.tensor_mul(sq[:, C:2 * C], oT1[:, c0:c0 + C], oT1[:, c0:c0 + C])
            ss = ps_lz.tile([1, C], f32, tag="lz")
            nc.tensor.matmul(ss[:], ones_col[:], sq[:, 0:C], start=True, stop=False)
            nc.tensor.matmul(ss[:], ones_col[:], sq[:, C:2 * C], start=False, stop=True)
            nc.vector.tensor_copy(ss_all[:, c0:c0 + C], ss[:])

        # ================= epilogue =================
        eps_t = cp.tile([1, 1], f32)
        nc.vector.memset(eps_t[:], EPS)
        nc.scalar.activation(ss_all[:], ss_all[:], AF.Sqrt, bias=eps_t[:, 0:1],
                             scale=1.0 / DK)
        rn_f = cp.tile([1, T], f32)
        nc.vector.reciprocal(rn_f[:], ss_all[:])
        nc.vector.tensor_copy(rnorm[:], rn_f[:])
        for j in range(T // 512):
            c0 = j * 512
            for gT_ in (gT0, gT1):
                sgt = wp.tile([C, 512], bf, tag="sgt")
                nc.scalar.activation(sgt[:], gT_[:, c0:c0 + 512], AF.Sigmoid)
                nc.vector.tensor_mul(gT_[:, c0:c0 + 512], gT_[:, c0:c0 + 512],
                                     sgt[:])
        # gating: og (reusing oT tiles) = oT * gnorm * rnorm_bcast * silu(g)
        for ch in range(NCH):
            c0 = ch * C
            rb = ps_st.tile([C, C], f32, tag="st")
            nc.tensor.matmul(rb[:], ones_row_b[:], rnorm[:, c0:c0 + C], **MMf)
            for hh, (oT_, gs_) in enumerate(((oT0, gT0), (oT1, gT1))):
                tmp = wp.tile([C, C], f32, tag="ogt")
                nc.vector.scalar_tensor_tensor(tmp[:], oT_[:, c0:c0 + C],
                                               gnw[:, hh:hh + 1], rb[:],
                                               ALU.mult, ALU.mult)
                nc.vector.tensor_mul(oT_[:, c0:c0 + C], tmp[:], gs_[:, c0:c0 + C])

        # o_proj: out[t, :] = og^T . wo ; per 128-row chunk
        for i in range(NCH):
            c0 = i * C
            for n in range(2):
                ps_o = ps_big.tile([C, 512], f32, tag="big")
                nc.tensor.matmul(ps_o[:], oT0[:, c0:c0 + C],
                                 wo[:, n * 512:(n + 1) * 512], start=True, stop=False)
                nc.tensor.matmul(ps_o[:], oT1[:, c0:c0 + C],
                                 wo[:, D + n * 512:D + (n + 1) * 512],
                                 start=False, stop=True)
                stg = op_.tile([C, 512], f32, tag="stage")
                nc.vector.tensor_copy(stg[:], ps_o[:])
                dma(out_d.ap()[c0:c0 + C, n * 512:(n + 1) * 512], stg[:])
    _legalize_waits(nc)
    return nc


def _get_nc():
    if "nc" not in _CACHE:
        _CACHE["nc"] = _build_nc()
    return _CACHE["nc"]


# ---------------------------------------------------------------- host side
def _shard_inputs(hidden_states, Wq, Wk, Wv, Wg, Wo, sk_w1, sk_w2, sk_b2,
                  sv_w1, sv_w2, sv_b2, gnorm_w):
    """Build the 8 per-core input maps. Core i -> (b = i//4, h = i%4)."""
    w1 = np.concatenate([sk_w1, sv_w1], axis=1)                     # [D, 2*LOW]
    in_maps = []
    for i in range(N_CORES):
        b, h = divmod(i, H)
        sl = slice(h * DK, (h + 1) * DK)
        sm = slice(h * M, (h + 1) * M)
        gnw = np.ascontiguousarray(gnorm_w.reshape(2, C).T)          # [128, 2]
        in_maps.append({
            "xT": np.ascontiguousarray(hidden_states[b].T).astype(BF16),
            "wq": np.ascontiguousarray(Wq[:, sl]).astype(BF16),
            "wk": np.ascontiguousarray(Wk[:, sl]).astype(BF16),
            "wv": np.ascontiguousarray(Wv[:, sl]).astype(BF16),
            "wg": np.ascontiguousarray(Wg[:, sl]).astype(BF16),
            "wo": np.ascontiguousarray(Wo[sl, :]).astype(BF16),
            "w1": np.ascontiguousarray(w1).astype(BF16),
            "w2k": np.ascontiguousarray(sk_w2[:, sm]).astype(BF16),
            "w2v": np.ascontiguousarray(sv_w2[:, sm]).astype(BF16),
            "bkr": np.ascontiguousarray(sk_b2[None, sm]).astype(BF16),
            "bvr": np.ascontiguousarray(sv_b2[None, sm]).astype(BF16),
            "bvc": np.ascontiguousarray(sv_b2[sm, None]).astype(np.float32),
            "gnw": np.ascontiguousarray(gnw).astype(np.float32),
        })
    return in_maps


def kernel(**inputs):
    from concourse.bass_utils import run_bass_kernel_spmd

    nc = _get_nc()
    in_maps = _shard_inputs(**{k: np.asarray(v) for k, v in inputs.items()})
    res = run_bass_kernel_spmd(nc, in_maps, core_ids=list(range(N_CORES)))
    _CACHE["last_results"] = res
    out = np.zeros((B, T, D), dtype=np.float32)
    for i in range(N_CORES):
        out[i // H] += res.results[i]["out"]
    return out


# revision 31
# speedup vs baseline: 1.1009x; 1.1009x over previous
"""ABC attention (Attention with Bounded-memory Control) on 8 TRN2 NeuronCores.

Sharding: one (batch, head) pair per core (B=2 x H=4 = 8 shards). Each core
computes its head's q/k/v/g projections, the chunked linear-attention ABC
recurrence (chunk C=128, [M,D] cumulative slot states instead of the O(T^2)
attention matrices), the gated RMSNorm epilogue, and its partial o_proj
contribution [T, D]. The host sums the 4 per-head partials of each batch.

Self-contained: hardcodes all shapes; inputs arrive as the full unsharded
tensors keyed as in setup_inputs().
"""
import contextlib

import numpy as np
import ml_dtypes

# ---------------------------------------------------------------- constants
B, T, D, H = 2, 2048, 1024, 4
DK = D // H            # 256 head dim
HALF = DK // 2         # 128 rope half
M = 64                 # slots
LOW = 16               # gate low rank
C = 128                # time chunk
NCH = T // C           # 16 chunks
CLAMP = 32.0
EPS = 1e-5
SCALE = DK ** -0.5
N_CORES = 8

BF16 = ml_dtypes.bfloat16

_CACHE = {}


# ---------------------------------------------------------------- tile patch
def _install_tile_patch():
    """This walrus build rejects >1 sync wait per instruction; the stock Tile
    tail drain carries one wait per live processor. Split the extra waits onto
    single-wait SP nops before the barrier + semaphore clear."""
    import concourse.tile as tile
    from concourse import mybir
    from concourse.vector_clock import ScopedClock

    if getattr(tile.TileContext, "_abc_patch", False):
        return
    tile.TileContext._abc_patch = True

    def _drain_and_barrier_split(self, tick_clock, wait_clock):
        drain_inst = self.nc.sync.drain()
        wait_clock.add_sem_waits(
            drain_inst.ins, ScopedClock({None: tick_clock.global_clock})
        )
        si = drain_inst.ins.sync_info
        waits = list(si.on_wait) if si is not None else []
        if len(waits) > 1:
            drain_inst.ins.sync_info = mybir.SyncInfo(on_wait=[waits[0]], on_update=[])
            for w in waits[1:]:
                nop = self.nc.sync.nop()
                nop.ins.sync_info = mybir.SyncInfo(on_wait=[w], on_update=[])
        self.nc.all_engine_barrier()
        assert self.sems is not None
        popped = self.nc._tile_sem_poison_stack.pop()
        assert popped is self._sem_poison
        self.nc.clear_and_free_semaphores(list(self.sems.allocated().values()))
        self.nc.all_engine_barrier()

    tile.TileContext._drain_and_barrier = _drain_and_barrier_split


def _legalize_waits(nc):
    """This walrus build accepts at most one sync wait per regular instruction
    (two for EventSemaphore). Hoist overflow waits onto fresh NoOps inserted
    just before the offending instruction in the same engine stream."""
    from concourse import mybir

    n = 0
    for blk in nc.m.functions[0].blocks:
        rebuilt = []
        changed = False
        for inst in blk.instructions:
            si = inst.sync_info
            waits = list(si.on_wait) if si is not None else []
            cap = 2 if isinstance(inst, mybir.InstEventSemaphore) else 1
            if len(waits) > cap:
                changed = True
                for w in waits[cap:]:
                    nop = mybir.InstNoOp(name=f"WSPLIT-{n}", ins=[], outs=[])
                    n += 1
                    nop.engine = inst.engine
                    nop.sync_info = mybir.SyncInfo(on_wait=[w], on_update=[])
                    rebuilt.append(nop)
                inst.sync_info = mybir.SyncInfo(on_wait=waits[:cap],
                                                on_update=list(si.on_update))
            rebuilt.append(inst)
        if changed:
            blk.instructions = rebuilt


# ---------------------------------------------------------------- host tables
def _rope_tables():
    inv_freq = 1.0 / (10000.0 ** (np.arange(HALF, dtype=np.float64) / HALF))
    ang = np.arange(T, dtype=np.float64)[:, None] * inv_freq[None, :]  # [T, HALF]
    cos = np.cos(ang).astype(np.float32)
    sin = np.sin(ang).astype(np.float32)
    # D-layout [HALF, T]: row i, col t
    cosT = np.ascontiguousarray(cos.T)
    sinT = np.ascontiguousarray(sin.T)
    # T-layout packed per chunk: [128, NCH*128]; (p, c*128+j) = table[c*128+p, j]
    cosn = np.ascontiguousarray(
        cos.reshape(NCH, C, HALF).transpose(1, 0, 2).reshape(C, NCH * HALF))
    sinn = np.ascontiguousarray(
        sin.reshape(NCH, C, HALF).transpose(1, 0, 2).reshape(C, NCH * HALF))
    return cosT, sinT, cosn, sinn


# ---------------------------------------------------------------- builder
def _build_nc():
    _install_tile_patch()
    import concourse.bass as bass
    import concourse.tile as tile
    from concourse import mybir

    f32 = mybir.dt.float32
    bf = mybir.dt.bfloat16
    AF = mybir.ActivationFunctionType
    ALU = mybir.AluOpType

    nc = bass.Bass("TRN2", target_bir_lowering=False, debug=False,
                   num_devices=N_CORES)

    # ---- dram parameters (per-core shards supplied via in_maps)
    xT_d = nc.dram_tensor("xT", [D, T], bf, kind="ExternalInput")
    wq_d = nc.dram_tensor("wq", [D, DK], bf, kind="ExternalInput")
    wk_d = nc.dram_tensor("wk", [D, DK], bf, kind="ExternalInput")
    wv_d = nc.dram_tensor("wv", [D, DK], bf, kind="ExternalInput")
    wg_d = nc.dram_tensor("wg", [D, DK], bf, kind="ExternalInput")
    wo_d = nc.dram_tensor("wo", [DK, D], bf, kind="ExternalInput")  # gnorm-scaled
    w1_d = nc.dram_tensor("w1", [D, 2 * LOW], bf, kind="ExternalInput")
    w2k_d = nc.dram_tensor("w2k", [LOW, M], bf, kind="ExternalInput")
    w2v_d = nc.dram_tensor("w2v", [LOW, M], bf, kind="ExternalInput")
    bkr_d = nc.dram_tensor("bkr", [1, M], bf, kind="ExternalInput")
    bvr_d = nc.dram_tensor("bvr", [1, M], bf, kind="ExternalInput")
    bvc_d = nc.dram_tensor("bvc", [M, 1], f32, kind="ExternalInput")
    out_d = nc.dram_tensor("out", [T, D], f32, kind="ExternalOutput")

    # ---- inline constants (same on every core)
    cosT_h, sinT_h, cosn_h, sinn_h = _rope_tables()
    mask_h = np.triu(np.ones((C, C), dtype=np.float32))   # [s,t] = 1 if t >= s
    ident_h = np.eye(C, dtype=np.float32)
    cosT_c = nc.inline_tensor(cosT_h.astype(BF16), name="cosT")
    sinT_c = nc.inline_tensor(sinT_h.astype(BF16), name="sinT")
    cosn_c = nc.inline_tensor(cosn_h.astype(BF16), name="cosn")
    sinn_c = nc.inline_tensor(sinn_h.astype(BF16), name="sinn")
    mask_f_c = nc.inline_tensor(mask_h, name="mask_f")
    mask_b_c = nc.inline_tensor(mask_h.astype(BF16), name="mask_b")
    ident_c = nc.inline_tensor(ident_h.astype(BF16), name="ident")
    ones_col_c = nc.inline_tensor(np.ones((C, 1), dtype=BF16), name="ones_col")
    ones_row_b_c = nc.inline_tensor(np.ones((1, C), dtype=BF16), name="ones_row_b")

    with tile.TileContext(nc) as tc, contextlib.ExitStack() as es:
        cp = es.enter_context(tc.tile_pool(name="const", bufs=1))
        wp = es.enter_context(tc.tile_pool(name="work", bufs=3))
        op_ = es.enter_context(tc.tile_pool(name="outstage", bufs=2))
        # PSUM: 8 banks total.  big=2 (projections / o_proj), st=2 (ST + state
        # K-deltas), lz=2 (logits+zk+zv, slot gates, z-deltas, ptT), ot=2
        # (oT halves + AT + ss row, V-deltas)
        ps_big = es.enter_context(tc.tile_pool(name="psbig", bufs=2, space="PSUM"))
        ps_st = es.enter_context(tc.tile_pool(name="psst", bufs=2, space="PSUM"))
        ps_lz = es.enter_context(tc.tile_pool(name="pslz", bufs=2, space="PSUM"))
        ps_ot = es.enter_context(tc.tile_pool(name="psot", bufs=2, space="PSUM"))

        dma = nc.sync.dma_start

        # ================= constant / persistent SBUF =================
        # small weights first so the first projections can start ASAP
        w_sb = {}
        for name, d_t in (("wq", wq_d), ("wk", wk_d), ("wg", wg_d)):
            t_ = cp.tile([C, 8 * DK], bf, tag=f"w_{name}")  # ktile kt at [kt*DK,)
            dma(t_[:].rearrange("p (n c) -> p n c", n=8),
                d_t.ap().rearrange("(n p) c -> p n c", p=C))
            w_sb[name] = t_
        # fused [wk | wv] per ktile for the natural-layout k/v projection
        wkv = cp.tile([C, 8 * 2 * DK], bf)
        dma(wkv[:].rearrange("p (n c) -> p n c", n=8)[:, :, 0:DK],
            wk_d.ap().rearrange("(n p) c -> p n c", p=C))
        dma(wkv[:].rearrange("p (n c) -> p n c", n=8)[:, :, DK:2 * DK],
            wv_d.ap().rearrange("(n p) c -> p n c", p=C))
        wo = cp.tile([C, 2 * D], bf)            # half hh at cols [hh*D, ...)
        dma(wo[:].rearrange("p (n c) -> p n c", n=2),
            wo_d.ap().rearrange("(n p) c -> p n c", p=C))
        w1 = cp.tile([C, 8 * 2 * LOW], bf)
        dma(w1[:].rearrange("p (n c) -> p n c", n=8),
            w1_d.ap().rearrange("(n p) c -> p n c", p=C))
        w2k = cp.tile([LOW, M], bf); dma(w2k[:], w2k_d.ap())
        w2v = cp.tile([LOW, M], bf); dma(w2v[:], w2v_d.ap())
        bkr = cp.tile([1, M], bf); dma(bkr[:], bkr_d.ap())
        bvr = cp.tile([1, M], bf); dma(bvr[:], bvr_d.ap())
        bvc = cp.tile([M, 1], f32); dma(bvc[:], bvc_d.ap())

        cosT = cp.tile([C, T], bf); dma(cosT[:], cosT_c.ap())
        sinT = cp.tile([C, T], bf); dma(sinT[:], sinT_c.ap())
        cosn = cp.tile([C, T], bf); dma(cosn[:], cosn_c.ap())
        sinn = cp.tile([C, T], bf); dma(sinn[:], sinn_c.ap())
        mask_f = cp.tile([C, C], f32); dma(mask_f[:], mask_f_c.ap())
        mask_b = cp.tile([C, C], bf); dma(mask_b[:], mask_b_c.ap())
        ident = cp.tile([C, C], bf); dma(ident[:], ident_c.ap())
        ones_col = cp.tile([C, 1], bf); dma(ones_col[:], ones_col_c.ap())
        ones_row_b = cp.tile([1, C], bf); dma(ones_row_b[:], ones_row_b_c.ap())

        # x^T in 8 separate DMAs so the first ktile lands early
        xT = cp.tile([C, 8 * T], bf)            # ktile kt at cols [kt*T, (kt+1)*T)
        for kt in range(8):
            dma(xT[:, kt * T:(kt + 1) * T], xT_d.ap()[kt * C:(kt + 1) * C, :])

        qT0 = cp.tile([C, T], bf); qT1 = cp.tile([C, T], bf)     # rope(q)^T halves
        kT0 = cp.tile([C, T], bf); kT1 = cp.tile([C, T], bf)     # rope(k)^T halves
        k_nat = cp.tile([C, NCH * DK], bf)     # rope(k) chunk tch at cols [tch*DK,)
        v_nat = cp.tile([C, NCH * DK], bf)
        gT0 = cp.tile([C, T], bf); gT1 = cp.tile([C, T], bf)     # -> silu in place
        yk = cp.tile([LOW, T], bf)
        yv = cp.tile([LOW, T], bf)
        sk_all = cp.tile([C, NCH * M], bf)     # clamped slot logits -> exp in place
        sv_all = cp.tile([C, NCH * M], bf)
        svT_all = cp.tile([M, T], bf)
        wk_e, wv_e, wvT_e = sk_all, sv_all, svT_all
        kst_pfx = cp.tile([C, NCH * C], bf)    # KstateT prefix: chunk c, half hh at
        #                                        cols [c*128 + hh*64)
        vst_pfx = cp.tile([M, NCH * DK], bf)   # Vstate prefix
        zk_pfx = cp.tile([1, NCH * M], bf)
        zv_pfx = cp.tile([1, NCH * M], bf)
        kst_acc = cp.tile([C, C], f32)
        vst_acc = cp.tile([M, DK], f32)
        z_acc = cp.tile([1, 2 * M], f32)
        oT0 = cp.tile([C, T], bf); oT1 = cp.tile([C, T], bf)     # -> gated in place
        ss2d = cp.tile([NCH, C], bf)           # sum(o^2): row ch = chunk ch
        rnT = cp.tile([C, NCH], f32)           # 1/sqrt(ms+eps), [t-in-chunk, chunk]

        MMf = dict(start=True, stop=True)

        def rope_pair(dst0, dst1, ps0, ps1, cos_t, sin_t, col, n, tagpfx):
            t1 = wp.tile([C, n], f32, tag=tagpfx + "A")
            t2 = wp.tile([C, n], f32, tag=tagpfx + "B")
            nc.vector.tensor_mul(t1[:], ps0, cos_t)
            nc.vector.tensor_mul(t2[:], ps1, sin_t)
            nc.vector.tensor_sub(dst0[:, col:col + n], t1[:], t2[:])
            nc.vector.tensor_mul(t1[:], ps1, cos_t)
            nc.vector.tensor_mul(t2[:], ps0, sin_t)
            nc.vector.tensor_add(dst1[:, col:col + n], t1[:], t2[:])

        # ================= phase A: projections =================
        # q^T, k^T, g^T (D-layout) + ykv, per 512-wide time slab
        for j in range(T // 512):
            c0 = j * 512
            ps_y = ps_big.tile([2 * LOW, 512], f32, tag="big")
            for kt in range(8):
                nc.tensor.matmul(ps_y[:], w1[:, kt * 2 * LOW:(kt + 1) * 2 * LOW],
                                 xT[:, kt * T + c0:kt * T + c0 + 512],
                                 start=(kt == 0), stop=(kt == 7))
            # partition-16 source reads are illegal on DVE; stage to SBUF then
            # shift partitions via SBUF->SBUF DMA
            ystg = wp.tile([2 * LOW, 512], bf, tag="ystg")
            nc.vector.tensor_copy(ystg[:], ps_y[:])
            dma(yk[0:LOW, c0:c0 + 512], ystg[0:LOW, :])
            dma(yv[0:LOW, c0:c0 + 512], ystg[LOW:2 * LOW, :])

            for wname, d0, d1, tg in (("wq", qT0, qT1, "rp"), ("wk", kT0, kT1, "rp")):
                w_ = w_sb[wname]
                psA = ps_big.tile([C, 512], f32, tag="big")
                psB = ps_big.tile([C, 512], f32, tag="big")
                for kt in range(8):
                    nc.tensor.matmul(psA[:], w_[:, kt * DK:kt * DK + HALF],
                                     xT[:, kt * T + c0:kt * T + c0 + 512],
                                     start=(kt == 0), stop=(kt == 7))
                for kt in range(8):
                    nc.tensor.matmul(psB[:], w_[:, kt * DK + HALF:(kt + 1) * DK],
                                     xT[:, kt * T + c0:kt * T + c0 + 512],
                                     start=(kt == 0), stop=(kt == 7))
                rope_pair(d0, d1, psA[:], psB[:],
                          cosT[:, c0:c0 + 512], sinT[:, c0:c0 + 512], c0, 512, tg)
            # g^T: no rope; silu applied right here (in place) so the sigmoid
            # activation table loads once, before the long Exp phase
            for hh, dst in ((0, gT0), (1, gT1)):
                psG = ps_big.tile([C, 512], f32, tag="big")
                for kt in range(8):
                    nc.tensor.matmul(psG[:], w_sb["wg"][:, kt * DK + hh * HALF:
                                                        kt * DK + (hh + 1) * HALF],
                                     xT[:, kt * T + c0:kt * T + c0 + 512],
                                     start=(kt == 0), stop=(kt == 7))
                nc.vector.tensor_copy(dst[:, c0:c0 + 512], psG[:])
                sgt = wp.tile([C, 512], bf, tag="sgt")
                nc.scalar.activation(sgt[:], dst[:, c0:c0 + 512], AF.Sigmoid)
                nc.vector.tensor_mul(dst[:, c0:c0 + 512], dst[:, c0:c0 + 512],
                                     sgt[:])

        # k, v in natural T-layout (fused [wk|wv] rhs), per 128 chunk
        for tch in range(NCH):
            c0 = tch * C
            psKV = ps_big.tile([C, 512], f32, tag="big")
            for kt in range(8):
                nc.tensor.matmul(psKV[:], xT[:, kt * T + c0:kt * T + c0 + C],
                                 wkv[:, kt * 2 * DK:(kt + 1) * 2 * DK],
                                 start=(kt == 0), stop=(kt == 7))
            tc_ = cosn[:, tch * HALF:(tch + 1) * HALF]
            ts_ = sinn[:, tch * HALF:(tch + 1) * HALF]
            t1 = wp.tile([C, HALF], f32, tag="rnA")
            t2 = wp.tile([C, HALF], f32, tag="rnB")
            nc.vector.tensor_mul(t1[:], psKV[:, 0:HALF], tc_)
            nc.vector.tensor_mul(t2[:], psKV[:, HALF:DK], ts_)
            nc.vector.tensor_sub(k_nat[:, tch * DK:tch * DK + HALF], t1[:], t2[:])
            nc.vector.tensor_mul(t1[:], psKV[:, HALF:DK], tc_)
            nc.vector.tensor_mul(t2[:], psKV[:, 0:HALF], ts_)
            nc.vector.tensor_add(k_nat[:, tch * DK + HALF:(tch + 1) * DK], t1[:], t2[:])
            nc.scalar.copy(v_nat[:, tch * DK:(tch + 1) * DK], psKV[:, DK:2 * DK])

        # ---- slot gate logits (clamped), then batched exp
        for tch in range(NCH):
            c0 = tch * C
            ps_s = ps_lz.tile([C, 2 * M], f32, tag="lz")
            nc.tensor.matmul(ps_s[:, 0:M], yk[:, c0:c0 + C], w2k[:],
                             start=True, stop=False)
            nc.tensor.matmul(ps_s[:, 0:M], ones_row_b[:], bkr[:],
                             start=False, stop=True)
            nc.tensor.matmul(ps_s[:, M:2 * M], yv[:, c0:c0 + C], w2v[:],
                             start=True, stop=False)
            nc.tensor.matmul(ps_s[:, M:2 * M], ones_row_b[:], bvr[:],
                             start=False, stop=True)
            nc.vector.tensor_scalar(sk_all[:, tch * M:(tch + 1) * M], ps_s[:, 0:M],
                                    -CLAMP, CLAMP, ALU.max, ALU.min)
            nc.vector.tensor_scalar(sv_all[:, tch * M:(tch + 1) * M], ps_s[:, M:2 * M],
                                    -CLAMP, CLAMP, ALU.max, ALU.min)
            ps_t = ps_lz.tile([M, C], f32, tag="lz")
            nc.tensor.matmul(ps_t[:], w2v[:], yv[:, c0:c0 + C], **MMf)
            svt_tmp = wp.tile([M, C], f32, tag="svtmp")
            nc.vector.tensor_scalar(svt_tmp[:], ps_t[:], bvc[:, 0:1], -CLAMP,
                                    ALU.add, ALU.max)
            nc.vector.tensor_scalar_min(svT_all[:, c0:c0 + C], svt_tmp[:], CLAMP)
        nc.scalar.activation(wk_e[:], sk_all[:], AF.Exp)
        nc.scalar.activation(wv_e[:], sv_all[:], AF.Exp)
        nc.scalar.activation(wvT_e[:], svT_all[:], AF.Exp)

        # ---- state prefixes (sequential over chunks)
        for tch in range(NCH):
            wk_c = wk_e[:, tch * M:(tch + 1) * M]
            wv_c = wv_e[:, tch * M:(tch + 1) * M]
            kd = ps_st.tile([C, C], f32, tag="st")
            for hh in range(2):
                nc.tensor.matmul(kd[:, hh * M:(hh + 1) * M],
                                 k_nat[:, tch * DK + hh * C:tch * DK + (hh + 1) * C],
                                 wk_c, **MMf)
            vd = ps_ot.tile([M, DK], f32, tag="ot")
            nc.tensor.matmul(vd[:], wv_c, v_nat[:, tch * DK:(tch + 1) * DK], **MMf)
            zd = ps_lz.tile([1, 2 * M], f32, tag="lz")
            nc.tensor.matmul(zd[:, 0:M], ones_col[:], wk_c, **MMf)
            nc.tensor.matmul(zd[:, M:2 * M], ones_col[:], wv_c, **MMf)
            if tch == 0:
                nc.vector.tensor_copy(kst_acc[:], kd[:])
                nc.vector.tensor_copy(vst_acc[:], vd[:])
                nc.vector.tensor_copy(z_acc[:], zd[:])
            else:
                nc.vector.tensor_add(kst_acc[:], kst_acc[:], kd[:])
                nc.vector.tensor_add(vst_acc[:], vst_acc[:], vd[:])
                nc.vector.tensor_add(z_acc[:], z_acc[:], zd[:])
            nc.vector.tensor_copy(kst_pfx[:, tch * C:(tch + 1) * C], kst_acc[:])
            nc.vector.tensor_copy(vst_pfx[:, tch * DK:(tch + 1) * DK], vst_acc[:])
            nc.vector.tensor_copy(zk_pfx[:, tch * M:(tch + 1) * M], z_acc[:, 0:M])
            nc.vector.tensor_copy(zv_pfx[:, tch * M:(tch + 1) * M], z_acc[:, M:2 * M])

        # ================= phase B: per-chunk attention =================
        for ch in range(NCH):
            c0 = ch * C
            wk_c = wk_e[:, ch * M:(ch + 1) * M]
            wv_c = wv_e[:, ch * M:(ch + 1) * M]
            # ST[s,t] = sum_d k^T[d,s] q^T[d,t]
            st_ps = ps_st.tile([C, C], f32, tag="st")
            nc.tensor.matmul(st_ps[:], kT0[:, c0:c0 + C], qT0[:, c0:c0 + C],
                             start=True, stop=False)
            nc.tensor.matmul(st_ps[:], kT1[:, c0:c0 + C], qT1[:, c0:c0 + C],
                             start=False, stop=True)
            st_m = wp.tile([C, C], bf, tag="stm")
            nc.vector.tensor_mul(st_m[:], st_ps[:], mask_f[:])

            lz = ps_lz.tile([C, 3 * M], f32, tag="lz")
            # logits[t,m] = ST_m^T wk + q_t . Kstate
            nc.tensor.matmul(lz[:, 0:M], st_m[:], wk_c, start=True, stop=(ch == 0))
            if ch > 0:
                pcol = (ch - 1) * C
                nc.tensor.matmul(lz[:, 0:M], qT0[:, c0:c0 + C],
                                 kst_pfx[:, pcol:pcol + M], start=False, stop=False)
                nc.tensor.matmul(lz[:, 0:M], qT1[:, c0:c0 + C],
                                 kst_pfx[:, pcol + M:pcol + 2 * M],
                                 start=False, stop=True)
            # zk, zv: within-chunk cumulative sums + carried state
            pm = (ch - 1) * M
            nc.tensor.matmul(lz[:, M:2 * M], mask_b[:], wk_c,
                             start=True, stop=(ch == 0))
            if ch > 0:
                nc.tensor.matmul(lz[:, M:2 * M], ones_row_b[:],
                                 zk_pfx[:, pm:pm + M], start=False, stop=True)
            nc.tensor.matmul(lz[:, 2 * M:3 * M], mask_b[:], wv_c,
                             start=True, stop=(ch == 0))
            if ch > 0:
                nc.tensor.matmul(lz[:, 2 * M:3 * M], ones_row_b[:],
                                 zv_pfx[:, pm:pm + M], start=False, stop=True)

            rzk = wp.tile([C, M], f32, tag="rzk")
            nc.vector.reciprocal(rzk[:], lz[:, M:2 * M])
            rzv = wp.tile([C, M], f32, tag="rzv")
            nc.vector.reciprocal(rzv[:], lz[:, 2 * M:3 * M])
            l1 = wp.tile([C, M], f32, tag="l1")
            nc.vector.tensor_mul(l1[:], lz[:, 0:M], rzk[:])
            # logits*scale are small (|l1*SCALE| < ~4): exp without max-subtract
            e = wp.tile([C, M], f32, tag="e")
            rs = wp.tile([C, 1], f32, tag="rs")
            nc.scalar.activation(e[:], l1[:], AF.Exp, scale=SCALE,
                                 accum_out=rs[:, 0:1])
            rden = wp.tile([C, 1], f32, tag="rden")
            nc.vector.reciprocal(rden[:], rs[:])
            ptmp = wp.tile([C, M], f32, tag="ptmp")
            nc.vector.tensor_scalar_mul(ptmp[:], e[:], rden[:, 0:1])
            pt_bf = wp.tile([C, M], bf, tag="ptbf")
            nc.vector.tensor_mul(pt_bf[:], ptmp[:], rzv[:])

            ptT_ps = ps_lz.tile([M, C], bf, tag="lz")
            nc.tensor.transpose(ptT_ps[:], pt_bf[:], ident[:])
            ptT = wp.tile([M, C], bf, tag="ptT")
            nc.vector.tensor_copy(ptT[:], ptT_ps[:])

            # one [128, 512] bank: oT0 | oT1 | AT | ss
            ot = ps_ot.tile([C, 4 * C], f32, tag="ot")
            nc.tensor.matmul(ot[:, 2 * C:3 * C], wvT_e[:, c0:c0 + C], ptT[:], **MMf)
            at_m = wp.tile([C, C], bf, tag="atm")
            nc.vector.tensor_mul(at_m[:], ot[:, 2 * C:3 * C], mask_f[:])

            for hh in range(2):
                nc.tensor.matmul(ot[:, hh * C:(hh + 1) * C],
                                 v_nat[:, ch * DK + hh * C:ch * DK + (hh + 1) * C],
                                 at_m[:], start=True, stop=(ch == 0))
                if ch > 0:
                    nc.tensor.matmul(ot[:, hh * C:(hh + 1) * C],
                                     vst_pfx[:, (ch - 1) * DK + hh * C:
                                             (ch - 1) * DK + (hh + 1) * C],
                                     ptT[:], start=False, stop=True)
            nc.scalar.copy(oT0[:, c0:c0 + C], ot[:, 0:C])
            nc.scalar.copy(oT1[:, c0:c0 + C], ot[:, C:2 * C])
            sq = wp.tile([C, 2 * C], bf, tag="sq")
            nc.vector.tensor_mul(sq[:, 0:C], oT0[:, c0:c0 + C], oT0[:, c0:c0 + C])
            nc.vector.tensor_mul(sq[:, C:2 * C], oT1[:, c0:c0 + C], oT1[:, c0:c0 + C])
            nc.tensor.matmul(ot[0:1, 3 * C:4 * C], ones_col[:], sq[:, 0:C],
                             start=True, stop=False)
            nc.tensor.matmul(ot[0:1, 3 * C:4 * C], ones_col[:], sq[:, C:2 * C],
                             start=False, stop=True)
            ss_stg = wp.tile([1, C], bf, tag="ssstg")
            nc.vector.tensor_copy(ss_stg[:], ot[0:1, 3 * C:4 * C])
            dma(ss2d[ch:ch + 1, :], ss_stg[:])
            # gate in place: og = oT * silu(g)   (rnorm folded into out staging)
            nc.vector.tensor_mul(oT0[:, c0:c0 + C], oT0[:, c0:c0 + C],
                                 gT0[:, c0:c0 + C])
            nc.vector.tensor_mul(oT1[:, c0:c0 + C], oT1[:, c0:c0 + C],
                                 gT1[:, c0:c0 + C])

        # ================= epilogue =================
        # transpose ss2d -> [t-in-chunk, chunk] on the PE, then rsqrt(ms+eps)
        ssT_ps = ps_lz.tile([C, NCH], bf, tag="lz")
        nc.tensor.transpose(ssT_ps[:], ss2d[:], ident[0:NCH, 0:NCH])
        eps_t = cp.tile([C, 1], f32)
        nc.vector.memset(eps_t[:], EPS)
        msT = cp.tile([C, NCH], f32)
        nc.scalar.activation(msT[:], ssT_ps[:], AF.Sqrt, bias=eps_t[:, 0:1],
                             scale=1.0 / DK)
        nc.vector.reciprocal(rnT[:], msT[:])

        # o_proj: out[t, :] = (og / sqrt(ms+eps))^T . wo ; per 128-row chunk
        for i in range(NCH):
            c0 = i * C
            stg = op_.tile([C, D], f32, tag="stage")
            for n in range(2):
                ps_o = ps_big.tile([C, 512], f32, tag="big")
                nc.tensor.matmul(ps_o[:], oT0[:, c0:c0 + C],
                                 wo[:, n * 512:(n + 1) * 512], start=True, stop=False)
                nc.tensor.matmul(ps_o[:], oT1[:, c0:c0 + C],
                                 wo[:, D + n * 512:D + (n + 1) * 512],
                                 start=False, stop=True)
                nc.vector.tensor_scalar_mul(stg[:, n * 512:(n + 1) * 512],
                                            ps_o[:], rnT[:, i:i + 1])
            dma(out_d.ap()[c0:c0 + C, :], stg[:])
    _legalize_waits(nc)
    return nc


def _get_nc():
    if "nc" not in _CACHE:
        _CACHE["nc"] = _build_nc()
    return _CACHE["nc"]


# ---------------------------------------------------------------- host side
def _shard_inputs(hidden_states, Wq, Wk, Wv, Wg, Wo, sk_w1, sk_w2, sk_b2,
                  sv_w1, sv_w2, sv_b2, gnorm_w):
    """Build the 8 per-core input maps. Core i -> (b = i//4, h = i%4)."""
    w1 = np.concatenate([sk_w1, sv_w1], axis=1)                     # [D, 2*LOW]
    in_maps = []
    for i in range(N_CORES):
        b, h = divmod(i, H)
        sl = slice(h * DK, (h + 1) * DK)
        sm = slice(h * M, (h + 1) * M)
        wo_scaled = Wo[sl, :] * gnorm_w[:, None]   # fold gated-RMSNorm weight
        in_maps.append({
            "xT": np.ascontiguousarray(hidden_states[b].T).astype(BF16),
            "wq": np.ascontiguousarray(Wq[:, sl]).astype(BF16),
            "wk": np.ascontiguousarray(Wk[:, sl]).astype(BF16),
            "wv": np.ascontiguousarray(Wv[:, sl]).astype(BF16),
            "wg": np.ascontiguousarray(Wg[:, sl]).astype(BF16),
            "wo": np.ascontiguousarray(wo_scaled).astype(BF16),
            "w1": np.ascontiguousarray(w1).astype(BF16),
            "w2k": np.ascontiguousarray(sk_w2[:, sm]).astype(BF16),
            "w2v": np.ascontiguousarray(sv_w2[:, sm]).astype(BF16),
            "bkr": np.ascontiguousarray(sk_b2[None, sm]).astype(BF16),
            "bvr": np.ascontiguousarray(sv_b2[None, sm]).astype(BF16),
            "bvc": np.ascontiguousarray(sv_b2[sm, None]).astype(np.float32),
        })
    return in_maps


def kernel(**inputs):
    from concourse.bass_utils import run_bass_kernel_spmd

    nc = _get_nc()
    in_maps = _shard_inputs(**{k: np.asarray(v) for k, v in inputs.items()})
    res = run_bass_kernel_spmd(nc, in_maps, core_ids=list(range(N_CORES)))
    _CACHE["last_results"] = res
    out = np.zeros((B, T, D), dtype=np.float32)
    for i in range(N_CORES):
        out[i // H] += res.results[i]["out"]
    return out
